# revision 1
# baseline (speedup 1.0000x reference)
"""Trainium2 Bass kernel for nn_DCFMBlock (4-direction selective-scan block).

Strategy
--------
Data-parallel over batch: core b processes batch b (B=8 = 8 cores), no
collectives.  Key algebraic restructurings (validated vs the jax reference
in proto.py at rel err ~8e-7):

  * The DisparityMerge keeps only ym[:,:,:,0,:] (stream-half 0 = the first
    L=2304 sequence positions) and the scan is causal -> the second half of
    every scan, and input stream s=2 entirely, are dead code.  All heavy
    stages run at half the reference's size.
  * softplus(x) = -log(sigmoid(-x));  dA = exp(dt*A) = exp(nA * logsig)
    with nA = exp(A_logs) = -A, so the big exp pass is a single fused
    ACTIVATE(Exp, scale=nA_col, in=log_sigmoid) per state column -- no
    separate dt*A materialization.
  * Depthwise 3x3 conv folded into in_proj: 9 PSUM-accumulated matmuls with
    spatially shifted rhs views and per-tap weights
    W_tap[e,c] = in_proj_w[e,c] * conv_w[s*DE+e, tap] (prepared in numpy).
  * Directions 2,3 are within-half time-reversals of dirs 0,1: per-position
    tensors are built in forward order and consumed via negative-step APs;
    scan outputs stay in scan order and are flipped once at the merge.
  * The recurrence is DVE tensor_tensor_scan (state = dA*state + b along the
    free axis, fp32 internal state), 16 independent n-scans per 128-channel
    tile, chained across t-chunks via `initial` APs.

Layouts: channels cc = g*192+e tiled as 6x128 partitions; sequence chunked
by T=384 (6 chunks); state index n in free-dim blocks of NG=4.
"""
import sys
import numpy as np

sys.path.insert(0, "/opt/trn_rl_repo")

import concourse.bass as bass  # noqa: E402
import concourse.bacc as bacc  # noqa: E402
import concourse.mybir as mybir  # noqa: E402
from concourse import tile  # noqa: E402
from concourse.bass_utils import run_bass_kernel_spmd  # noqa: E402

F32 = mybir.dt.float32
BF16 = mybir.dt.bfloat16
AF = mybir.ActivationFunctionType
OP = mybir.AluOpType

B, S, H, W, DM = 8, 3, 48, 48, 96
DE, N, R, G = 192, 16, 6, 4
L = H * W            # 2304
NT = 2 * L           # tokens over streams 0,1
EPS = 1e-5
T = 768              # scan chunk length
K = L // T           # 6 chunks
NG = 4               # state columns per scan group
NGRP = N // NG       # 4 groups
CK = [(0, 480), (480, 960), (960, 1440), (1440, 1920), (1920, 2304)]

# dtype knobs (perf iteration)
DT_B = BF16   # b = dt*u*B scan input
DT_H = BF16   # scan output h
DT_HC = BF16  # h*C product
DT_DA = BF16  # exp output

# channel-tile pieces: TILE_PIECES[j] = [(row0, nrows, g, e0)]
TILE_PIECES = []
for _j in range(6):
    _lo, _hi = 128 * _j, 128 * (_j + 1)
    _pieces = []
    for _g in range(_lo // DE, (_hi - 1) // DE + 1):
        _a, _b = max(_lo, _g * DE), min(_hi, (_g + 1) * DE)
        _pieces.append((_a - _lo, _b - _a, _g, _a - _g * DE))
    TILE_PIECES.append(_pieces)

# xf-pieces: dir pieces additionally split at the 96-channel xf tile boundary
# XF_PIECES[j] = [(row0, nrows, g, s, et, er0)]
XF_PIECES = []
for _j in range(6):
    _pieces = []
    for (_r0, _nr, _g, _e0) in TILE_PIECES[_j]:
        _e = _e0
        while _e < _e0 + _nr:
            _et = _e // 96
            _take = min((_et + 1) * 96, _e0 + _nr) - _e
            _pieces.append((_r0 + (_e - _e0), _take, _g, _g % 2, _et, _e - _et * 96))
            _e += _take
    XF_PIECES.append(_pieces)


def _make_selbc():
    # rows 32:64 broadcast B rows, 64:96 C rows; first 16 k-rows select the
    # even direction of the pair, next 16 the odd.  Column block
    # jt*2048 + n*128 + m with jt = j%3: jt0 all-even rows, jt1 rows<64
    # even / rows>=64 odd, jt2 all-odd.
    sel = np.zeros((96, 3 * 16 * 128), np.float32)
    for jt in range(3):
        even = np.ones(128, bool)
        if jt == 1:
            even[64:] = False
        elif jt == 2:
            even[:] = False
        for n in range(16):
            c0 = jt * 2048 + n * 128
            for base in (32, 64):
                sel[base + n, c0:c0 + 128][even] = 1.0
                sel[base + 16 + n, c0:c0 + 128][~even] = 1.0
    return sel


def _prep_weights(inputs):
    """All layout preprocessing in numpy so device DMAs are plain [P, F]."""
    f = lambda k: np.ascontiguousarray(np.asarray(inputs[k], np.float32))
    in_proj_w = f('in_proj_w')            # (192, 96)
    conv_w = f('conv_w')                  # (576,1,3,3)
    conv_b = f('conv_b')
    x_proj_w = f('x_proj_w')              # (4,38,192)
    dt_projs_w = f('dt_projs_w')          # (4,192,6)
    dt_projs_b = f('dt_projs_b')          # (4,192)
    A_logs = f('A_logs')                  # (768,16)
    Ds = f('Ds')                          # (768,)

    # conv+proj fused weights per U-tile piece: U0 = s0/e0:128, U1 =
    # s0/e128:192 (rows 0:64) + s1/e0:64 (rows 64:128), U2 = s1/e64:192.
    # UPIECES: (utile, s, e0, out_base, width)
    wtapU = np.zeros((4, 9, 96, 128), np.float32)
    for pi, (ut, s, e0, ob, wd) in enumerate(UPIECES):
        for ky in range(3):
            for kx in range(3):
                tap = ky * 3 + kx
                e_glob = np.arange(wd) + e0
                wt = in_proj_w[e_glob] * conv_w[s * DE + e_glob, 0, ky, kx][:, None]
                wtapU[pi, tap, :, :wd] = wt.T                 # lhsT [c, e-piece]
    convbU = np.zeros((128, 3), np.float32)          # bias col per U-tile
    for (ut, s, e0, ob, wd) in UPIECES:
        convbU[ob:ob + wd, ut] = conv_b[s * DE + e0: s * DE + e0 + wd]

    # x_dbl lhsT blocks per direction: two k-pieces matching U-tile rows
    xprojU = np.zeros((4, 2, 128, 38), np.float32)
    for g in range(4):
        if g % 2 == 0:
            xprojU[g, 0, 0:128] = x_proj_w[g, :, 0:128].T
            xprojU[g, 1, 0:64] = x_proj_w[g, :, 128:192].T
        else:
            xprojU[g, 0, 64:128] = x_proj_w[g, :, 0:64].T
            xprojU[g, 1, 0:128] = x_proj_w[g, :, 64:192].T

    cc_g = np.arange(768) // DE
    cc_e = np.arange(768) % DE
    # dt lhsT over paired dts_lr rows: k 0:6 even-dir taps, 6:12 odd-dir
    dtw12 = np.zeros((12, 768), np.float32)
    for r in range(6):
        ev = (cc_g % 2 == 0)
        dtw12[r, ev] = dt_projs_w[cc_g[ev], cc_e[ev], r]
        dtw12[r + 6, ~ev] = dt_projs_w[cc_g[~ev], cc_e[~ev], r]
    dtb = dt_projs_b[cc_g, cc_e].reshape(6, 128).T.copy()       # (128, 6)
    mA = (-np.exp(A_logs)).reshape(6, 128, 16).transpose(1, 0, 2).reshape(128, 96).copy()
    Ds_np = Ds.reshape(6, 128).T.copy()              # (128, 6)

    opwT = np.stack([f('out_proj_w0').T, f('out_proj_w1').T])    # (2,192,96)
    # out_proj lhsT per merged-y piece (YPIECES[k01]: (prow0, nrows, e0))
    opwU = np.zeros((2, 2, 128, 96), np.float32)
    for k01 in range(2):
        for pi, (pr0, nr, e0) in enumerate(YPIECES[k01]):
            opwU[k01, pi, pr0:pr0 + nr] = opwT[k01][e0:e0 + nr]
    cabwT = np.stack([f('cab_w0').T, f('cab_w1').T])             # (2,96,96)
    cabb = np.stack([f('cab_b0'), f('cab_b1')], 1)               # (96,2)
    return dict(
        wtapU=wtapU.transpose(2, 0, 1, 3).reshape(96, 36 * 128),
        convbU=convbU,
        xprojU=xprojU.transpose(2, 0, 1, 3).reshape(128, 8 * 38),
        dtw12=dtw12, dtb=dtb, mA=mA, Ds=Ds_np,
        opwU=opwU.transpose(2, 0, 1, 3).reshape(128, 4 * 96),
        cabwT=cabwT.transpose(1, 0, 2).reshape(96, 192), cabb=cabb,
        eye=np.eye(128, dtype=np.float32), ones=np.ones((128, 1), np.float32),
        ones_row=np.ones((1, 128), np.float32), selBC=_make_selbc())


WSPECS = [
    ('wtapU', (96, 36 * 128)), ('convbU', (128, 3)), ('xprojU', (128, 8 * 38)),
    ('dtw12', (12, 768)), ('dtb', (128, 6)), ('mA', (128, 96)),
    ('Ds', (128, 6)), ('opwU', (128, 4 * 96)), ('cabwT', (96, 192)),
    ('cabb', (96, 2)), ('eye', (128, 128)), ('ones', (128, 1)),
    ('ones_row', (1, 128)), ('selBC', (96, 6144)),
]

# U-tile conv pieces: (utile, stream, e0, out_base_partition, width)
UPIECES = [(0, 0, 0, 0, 128), (1, 0, 128, 0, 64), (1, 1, 0, 64, 64),
           (2, 1, 64, 0, 128)]
# merged-y pieces per output stream: (partition_row0, nrows, e0)
YPIECES = [[(0, 128, 0), (0, 64, 128)],     # stream 0: tiles A, B
           [(64, 64, 0), (0, 128, 64)]]     # stream 1: tiles C, D
# merge sources per stream: (piece_idx, fwd_ytile, rev_ytile)
YMERGE = [[(0, 0, 3), (1, 1, 4)], [(0, 1, 4), (1, 2, 5)]]


BF16_W = {'selBC', 'wtapU', 'xprojU', 'dtw12', 'eye'}


def build_program():
    nc = bacc.Bacc("TRN2", target_bir_lowering=False, debug=False)
    xin = nc.dram_tensor('xin', [NT, DM], F32, kind='ExternalInput').ap()
    wap = {nm: nc.dram_tensor(nm, list(sh), BF16 if nm in BF16_W else F32,
                              kind='ExternalInput').ap()
           for nm, sh in WSPECS}
    yout = nc.dram_tensor('yout', [NT, DM], F32, kind='ExternalOutput').ap()
    import os
    reps = int(os.environ.get('BENCH_REPS', '1'))
    with tile.TileContext(nc) as tc:
        if reps > 1:
            with tc.For_i(0, reps, 1):
                _emit(tc, nc, xin, wap, yout)
        else:
            _emit(tc, nc, xin, wap, yout)
    nc.compile()
    return nc


def _emit(tc, nc, xin, wap, yout):
    from contextlib import ExitStack
    with ExitStack() as ctx:
        cpool = ctx.enter_context(tc.tile_pool(name='consts', bufs=1))
        xfpool = ctx.enter_context(tc.tile_pool(name='persist', bufs=1))

        # ---- constants ----
        sb = {}
        for nm, sh in WSPECS:
            t = cpool.tile(list(sh), BF16 if nm in BF16_W else F32, tag=nm,
                           name=nm)
            nc.sync.dma_start(t[:], wap[nm][:])
            sb[nm] = t
        wtapU_t = [sb['wtapU'][:, 128 * i:128 * (i + 1)] for i in range(36)]
        xprojU_t = [sb['xprojU'][:, 38 * i:38 * (i + 1)] for i in range(8)]
        opwU_t = [sb['opwU'][:, 96 * i:96 * (i + 1)] for i in range(4)]
        cabwT_t = [sb['cabwT'][:, 96 * i:96 * (i + 1)] for i in range(2)]

        # ---- persistent activations ----
        U = [xfpool.tile([128, L], BF16, tag=f'U{i}', name=f'U{i}') for i in range(3)]
        xpair = [xfpool.tile([96, L], BF16, tag=f'xp{p}', name=f'xp{p}') for p in range(2)]
        ytile = [xfpool.tile([128, L], BF16, tag=f'y{j}', name=f'y{j}') for j in range(6)]

        # ============ stage 1: LN + transpose -> hT [96, 4608] ==============
        with tc.tile_pool(name='st1', bufs=1) as p1, \
             tc.tile_pool(name='st1w', bufs=3) as p1w, \
             tc.tile_pool(name='ps1', bufs=3, space='PSUM') as ps1:
            hT = p1.tile([96, NT], BF16, tag='hT', name='hT')
            xt_all = p1.tile([128, NT // 128, DM], F32, tag='xt_all', name='xt_all')
            nc.gpsimd.dma_start(
                xt_all[:], xin.rearrange('(a p) c -> p a c', p=128))
            for ti in range(NT // 128):
                xt = xt_all[:, ti, :]
                rs = p1w.tile([128, 1], F32, tag='rs', name='rs')
                nc.vector.reduce_sum(rs[:], xt[:], axis=mybir.AxisListType.X)
                nm = p1w.tile([128, 1], F32, tag='nm', name='nm')
                nc.vector.tensor_scalar_mul(nm[:], rs[:], -1.0 / DM)
                xc = p1w.tile([128, DM], F32, tag='xc', name='xc')
                nc.vector.tensor_scalar_add(xc[:], xt, nm[:])
                sq = p1w.tile([128, DM], F32, tag='sq', name='sq')
                nc.vector.tensor_mul(sq[:], xc[:], xc[:])
                vs = p1w.tile([128, 1], F32, tag='vs', name='vs')
                nc.vector.reduce_sum(vs[:], sq[:], axis=mybir.AxisListType.X)
                veps = p1w.tile([128, 1], F32, tag='veps', name='veps')
                nc.vector.tensor_scalar(veps[:], vs[:], 1.0 / DM, EPS,
                                        op0=OP.mult, op1=OP.add)
                sd = p1w.tile([128, 1], F32, tag='sd', name='sd')
                nc.scalar.activation(sd[:], veps[:], AF.Ln)
                rstd = p1w.tile([128, 1], F32, tag='rstd', name='rstd')
                nc.scalar.activation(rstd[:], sd[:], AF.Exp, scale=-0.5)
                xn = p1w.tile([128, DM], BF16, tag='xn', name='xn')
                nc.vector.tensor_scalar_mul(xn[:], xc[:], rstd[:])
                pt = ps1.tile([96, 128], BF16, tag='pt', name='pt')
                nc.tensor.transpose(pt[:], xn[:], sb['eye'][:])
                nc.vector.tensor_copy(hT[:, 128 * ti:128 * (ti + 1)], pt[:])

            # ============ stage 2: fused conv+proj+bias+silu -> U tiles ======
            with tc.tile_pool(name='ps2', bufs=2, space='PSUM') as ps2:
                hT_img = [hT[:, s * L:(s + 1) * L].rearrange('p (h w) -> p h w', h=H)
                          for s in range(2)]
                taps = [(0, 0)] + [(dy, dx) for dy in (-1, 0, 1)
                                   for dx in (-1, 0, 1) if (dy, dx) != (0, 0)]
                for ut in range(3):
                    pieces = [(pi, p) for pi, p in enumerate(UPIECES) if p[0] == ut]
                    for (l0, l1) in CK:
                        h0 = l0 // W
                        pc = ps2.tile([128, l1 - l0], F32, tag='convps', name='convps')
                        pc_img = pc.rearrange('p (h w) -> p h w', w=W)
                        for pi, (ut_, s, e0, ob, wd) in pieces:
                            for i, (dy, dx) in enumerate(taps):
                                oh0 = max(h0, -dy)
                                oh1 = min(l1 // W, H - dy)
                                ow0, ow1 = max(0, -dx), min(W, W - dx)
                                if oh0 >= oh1:
                                    continue
                                tap = (dy + 1) * 3 + (dx + 1)
                                nc.tensor.matmul(
                                    pc_img[ob:ob + wd, oh0 - h0:oh1 - h0, ow0:ow1],
                                    wtapU_t[pi * 9 + tap][:, 0:wd],
                                    hT_img[s][:, oh0 + dy:oh1 + dy, ow0 + dx:ow1 + dx],
                                    start=(i == 0), stop=(i == len(taps) - 1))
                        nc.scalar.activation(
                            U[ut][:, l0:l1], pc[:], AF.Silu,
                            bias=sb['convbU'][:, ut:ut + 1])

            # ============ stage 3: x_dbl per direction (forward order) =======
            # xdbl rows: 0:6 dts_lr, 32:48 Bs, 64:80 Cs (PE-legal bases)
            with tc.tile_pool(name='ps3', bufs=2, space='PSUM') as ps3:
                for g in range(4):
                    if g % 2 == 0:
                        kpieces = [(U[0][:, :], xprojU_t[2 * g], 0, 128),
                                   (U[1][0:64, :], xprojU_t[2 * g + 1], 0, 64)]
                    else:
                        kpieces = [(U[1][64:128, :], xprojU_t[2 * g], 64, 64),
                                   (U[2][:, :], xprojU_t[2 * g + 1], 0, 128)]
                    for (l0, l1) in CK:
                        px = ps3.tile([80, l1 - l0], F32, tag='xdblps', name='xdblps')
                        for (mr0, mr1, wc0, wc1) in ((0, 6, 0, 6), (32, 48, 6, 22),
                                                     (64, 80, 22, 38)):
                            for ki, (uap, wt, kb, kn) in enumerate(kpieces):
                                nc.tensor.matmul(
                                    px[mr0:mr1, :],
                                    wt[kb:kb + kn, wc0:wc1],
                                    uap[:, l0:l1],
                                    start=(ki == 0), stop=(ki == 1))
                        if g % 2 == 0:
                            nc.scalar.copy(xpair[g // 2][0:6, l0:l1], px[0:6])
                            nc.scalar.copy(xpair[g // 2][32:48, l0:l1], px[32:48])
                            nc.scalar.copy(xpair[g // 2][64:80, l0:l1], px[64:80])
                        else:
                            stg = p1w.tile([80, l1 - l0], BF16, tag='xstg',
                                           name='xstg')
                            nc.scalar.copy(stg[0:6, :], px[0:6])
                            nc.scalar.copy(stg[32:48, :], px[32:48])
                            nc.scalar.copy(stg[64:80, :], px[64:80])
                            nc.gpsimd.dma_start(xpair[g // 2][6:12, l0:l1],
                                                stg[0:6, :])
                            nc.gpsimd.dma_start(xpair[g // 2][48:64, l0:l1],
                                                stg[32:48, :])
                            nc.gpsimd.dma_start(xpair[g // 2][80:96, l0:l1],
                                                stg[64:80, :])

        # ============ stage 5: selective scan ===============================
        carry = {}
        with tc.tile_pool(name='sc_sm', bufs=2) as psm, \
             tc.tile_pool(name='sc_yp', bufs=4) as pyp, \
             tc.tile_pool(name='sc_big', bufs=2) as pbig, \
             tc.tile_pool(name='sc_car', bufs=2) as pcar, \
             tc.tile_pool(name='ps5', bufs=2, space='PSUM') as ps5:
            for k in range(K):
                for j in range(6):
                    rev = TILE_PIECES[j][0][2] >= 2
                    cf = (K - 1 - k) if rev else k
                    fl0 = cf * T
                    # ---- dts -> softplus = ln(1 + exp(dts+b)) (fwd order) ----
                    pd = ps5.tile([128, T], F32, tag='dtsps', name='dtsps', bufs=1)
                    for (c0, c1) in ((0, 512), (512, T)):
                        nc.tensor.matmul(
                            pd[:, c0:c1],
                            sb['dtw12'][:, 128 * j:128 * (j + 1)],
                            xpair[j // 3][0:12, fl0 + c0:fl0 + c1])
                    e1 = psm.tile([128, T], F32, tag='e1', name='e1')
                    nc.scalar.activation(e1[:], pd[:], AF.Exp,
                                         bias=sb['dtb'][:, j:j + 1])
                    dt = psm.tile([128, T], F32, tag='dt', name='dt')
                    nc.scalar.activation(dt[:], e1[:], AF.Ln, bias=1.0)
                    dtu = psm.tile([128, T], F32, tag='dtu', name='dtu')
                    nc.vector.tensor_mul(dtu[:], dt[:], U[j % 3][:, fl0:fl0 + T])

                    def rv(ap2d):
                        return ap2d[:, ::-1] if rev else ap2d

                    ypart = []
                    for ng in range(NGRP):
                        dA = pbig.tile([128, NG * T], DT_DA, tag='dA', name='dA')
                        bt = pbig.tile([128, NG * T], DT_B, tag='bt', name='bt')
                        for nl in range(NG):
                            n = ng * NG + nl
                            nc.scalar.activation(
                                dA[:, nl * T:(nl + 1) * T], rv(dt[:]), AF.Exp,
                                scale=sb['mA'][:, 16 * j + n:16 * j + n + 1])
                            bexp = ps5.tile([128, T], F32, tag='bexp', name='bexp', bufs=2)
                            sc0 = (j % 3) * 2048 + n * 128
                            for (c0, c1) in ((0, 512), (512, T)):
                                nc.tensor.matmul(
                                    bexp[:, c0:c1],
                                    sb['selBC'][32:64, sc0:sc0 + 128],
                                    xpair[j // 3][32:64, fl0 + c0:fl0 + c1])
                            nc.vector.tensor_mul(
                                bt[:, nl * T:(nl + 1) * T], rv(dtu[:]), rv(bexp[:]))
                        ht = pbig.tile([128, NG * T], DT_H, tag='ht', name='ht')
                        for nl in range(NG):
                            init = 0.0 if k == 0 else carry[(j, ng)][:, nl:nl + 1]
                            nc.vector.tensor_tensor_scan(
                                ht[:, nl * T:(nl + 1) * T],
                                dA[:, nl * T:(nl + 1) * T],
                                bt[:, nl * T:(nl + 1) * T],
                                init, op0=OP.mult, op1=OP.add)
                        cnew = pcar.tile([128, NG], F32, tag=f'car{j}_{ng}', name=f'car{j}_{ng}')
                        nc.vector.tensor_copy(cnew[:], ht[:, T - 1::T])
                        carry[(j, ng)] = cnew
                        hc = pbig.tile([128, NG * T], DT_HC, tag='dA', name='hc')
                        for nl in range(NG):
                            n = ng * NG + nl
                            cexp = ps5.tile([128, T], F32, tag='cexp', name='cexp', bufs=1)
                            sc0 = (j % 3) * 2048 + n * 128
                            for (c0, c1) in ((0, 512), (512, T)):
                                nc.tensor.matmul(
                                    cexp[:, c0:c1],
                                    sb['selBC'][64:96, sc0:sc0 + 128],
                                    xpair[j // 3][64:96, fl0 + c0:fl0 + c1])
                            nc.vector.tensor_mul(
                                hc[:, nl * T:(nl + 1) * T],
                                ht[:, nl * T:(nl + 1) * T], rv(cexp[:]))
                        t1 = pyp.tile([128, T], BF16, tag='tr1', name='t1',
                                      bufs=2)
                        nc.gpsimd.tensor_add(t1[:], hc[:, 0:T], hc[:, T:2 * T])
                        t2 = pyp.tile([128, T], BF16, tag='tr2', name='t2',
                                      bufs=2)
                        nc.gpsimd.tensor_add(t2[:], hc[:, 2 * T:3 * T],
                                             hc[:, 3 * T:4 * T])
                        yp = pyp.tile([128, T], BF16, tag='yp', name='yp')
                        nc.gpsimd.tensor_add(yp[:], t1[:], t2[:])
                        ypart.append(yp)
                    ya = psm.tile([128, T], BF16, tag='ya', name='ya')
                    yb = psm.tile([128, T], BF16, tag='yb', name='yb')
                    ysum = psm.tile([128, T], F32, tag='ysum', name='ysum')
                    nc.vector.tensor_add(ya[:], ypart[0][:], ypart[1][:])
                    nc.vector.tensor_add(yb[:], ypart[2][:], ypart[3][:])
                    nc.vector.tensor_add(ysum[:], ya[:], yb[:])
                    uv = U[j % 3][:, ::-1] if rev else U[j % 3][:, :]
                    nc.vector.scalar_tensor_tensor(
                        ytile[j][:, k * T:(k + 1) * T],
                        uv[:, k * T:(k + 1) * T], sb['Ds'][:, j:j + 1], ysum[:],
                        op0=OP.mult, op1=OP.add)

        # ============ stage 6: merge + out_norm + out_proj + CAB ============
        # merged y per stream lives in two piece-tiles at the same partition
        # rows as their ytile sources (DVE cannot shift partitions); all
        # cross-partition moves are PE matmuls at legal bases.
        with tc.tile_pool(name='st6', bufs=1) as p6, \
             tc.tile_pool(name='st6w', bufs=2) as p6w, \
             tc.tile_pool(name='ps6', bufs=2, space='PSUM') as ps6, \
             tc.tile_pool(name='ps6b', bufs=1, space='PSUM') as ps6b:
            for k01 in range(2):
                xs = p6.tile([96, L], F32, tag='xs', name='xs')
                for (l0, l1) in CK:
                    w = l1 - l0
                    ymp = []
                    for pi, (pr0, nr, e0) in enumerate(YPIECES[k01]):
                        _, fj, rj = YMERGE[k01][pi]
                        t = p6w.tile([128, w], F32, tag=f'ym{pi}', name=f'ym{pi}')
                        nc.vector.tensor_add(
                            t[pr0:pr0 + nr, :],
                            ytile[fj][pr0:pr0 + nr, l0:l1],
                            ytile[rj][pr0:pr0 + nr, ::-1][:, l0:l1])
                        ymp.append(t)
                    sq = []
                    for pi, (pr0, nr, e0) in enumerate(YPIECES[k01]):
                        t = p6w.tile([128, w], F32, tag=f'sq{pi}', name=f'sq{pi}')
                        nc.vector.tensor_mul(t[pr0:pr0 + nr, :],
                                             ymp[pi][pr0:pr0 + nr, :],
                                             ymp[pi][pr0:pr0 + nr, :])
                        sq.append(t)
                    pst = ps6b.tile([1, w], F32, tag='statsps', name='statsps')
                    pst2 = ps6b.tile([1, w], F32, tag='statsps2', name='statsps2')
                    np_ = len(YPIECES[k01])
                    for pi, (pr0, nr, e0) in enumerate(YPIECES[k01]):
                        nc.tensor.matmul(pst[:], sb['ones'][pr0:pr0 + nr],
                                         ymp[pi][pr0:pr0 + nr, :],
                                         start=(pi == 0), stop=(pi == np_ - 1))
                        nc.tensor.matmul(pst2[:], sb['ones'][pr0:pr0 + nr],
                                         sq[pi][pr0:pr0 + nr, :],
                                         start=(pi == 0), stop=(pi == np_ - 1))
                    mrow = p6w.tile([1, w], F32, tag='mrow', name='mrow')
                    nc.vector.tensor_scalar_mul(mrow[:], pst[:], 1.0 / DE)
                    veps = p6w.tile([1, w], F32, tag='veps6', name='veps6')
                    nc.vector.tensor_scalar(veps[:], pst2[:], 1.0 / DE, EPS,
                                            op0=OP.mult, op1=OP.add)
                    m2 = p6w.tile([1, w], F32, tag='m2', name='m2')
                    nc.vector.tensor_mul(m2[:], mrow[:], mrow[:])
                    vr = p6w.tile([1, w], F32, tag='vr', name='vr')
                    nc.vector.tensor_sub(vr[:], veps[:], m2[:])
                    sdr = p6w.tile([1, w], F32, tag='sdr', name='sdr')
                    nc.scalar.activation(sdr[:], vr[:], AF.Ln)
                    rstd = p6w.tile([1, w], F32, tag='rstdr', name='rstdr')
                    nc.scalar.activation(rstd[:], sdr[:], AF.Exp, scale=-0.5)
                    mexp = ps6.tile([128, w], F32, tag='mexpps', name='mexpps',
                                    bufs=1)
                    nc.tensor.matmul(mexp[:], sb['ones_row'][:], mrow[:])
                    rexp = ps6.tile([128, w], F32, tag='rexpps', name='rexpps',
                                    bufs=1)
                    nc.tensor.matmul(rexp[:], sb['ones_row'][:], rstd[:])
                    po = ps6.tile([96, w], F32, tag='oprojps', name='oprojps')
                    for pi, (pr0, nr, e0) in enumerate(YPIECES[k01]):
                        yn = p6w.tile([128, w], F32, tag=f'yn{pi}', name=f'yn{pi}')
                        nc.vector.tensor_sub(yn[pr0:pr0 + nr, :],
                                             ymp[pi][pr0:pr0 + nr, :],
                                             mexp[pr0:pr0 + nr, :])
                        nc.vector.tensor_mul(yn[pr0:pr0 + nr, :],
                                             yn[pr0:pr0 + nr, :],
                                             rexp[pr0:pr0 + nr, :])
                        nc.tensor.matmul(po[:], opwU_t[k01 * 2 + pi][pr0:pr0 + nr],
                                         yn[pr0:pr0 + nr, :],
                                         start=(pi == 0), stop=(pi == np_ - 1))
                    nc.scalar.activation(xs[:, l0:l1], po[:], AF.Silu)
                av = p6w.tile([96, 1], F32, tag='av', name='av')
                nc.vector.reduce_sum(av[:], xs[:], axis=mybir.AxisListType.X)
                av2 = p6w.tile([96, 1], F32, tag='av2', name='av2')
                nc.vector.tensor_scalar_mul(av2[:], av[:], 1.0 / L)
                mx = p6w.tile([96, 1], F32, tag='mx', name='mx')
                nc.vector.reduce_max(mx[:], xs[:], axis=mybir.AxisListType.X)
                pg = ps6b.tile([96, 2], F32, tag='cabps', name='cabps')
                nc.tensor.matmul(pg[:, 0:1], cabwT_t[k01][:], av2[:])
                nc.tensor.matmul(pg[:, 1:2], cabwT_t[k01][:], mx[:])
                rr = p6w.tile([96, 2], F32, tag='rr', name='rr')
                nc.scalar.activation(rr[:], pg[:], AF.Relu,
                                     bias=sb['cabb'][:, k01:k01 + 1])
                gs = p6w.tile([96, 1], F32, tag='gs', name='gs')
                nc.vector.tensor_add(gs[:], rr[:, 0:1], rr[:, 1:2])
                ca = p6w.tile([96, 1], F32, tag='ca', name='ca')
                nc.scalar.activation(ca[:], gs[:], AF.Sigmoid)
                ca1 = p6w.tile([96, 1], F32, tag='ca1', name='ca1')
                nc.vector.tensor_scalar_add(ca1[:], ca[:], 1.0)
                ot_all = p6.tile([128, L // 128, DM], F32, tag='ot_all',
                                 name='ot_all')
                for sl in range(L // 128):
                    ocs = p6w.tile([96, 128], BF16, tag='ocs', name='ocs')
                    nc.vector.tensor_scalar_mul(ocs[:], xs[:, 128 * sl:128 * (sl + 1)],
                                                ca1[:])
                    ptr = ps6.tile([128, 96], BF16, tag='trps', name='trps', bufs=1)
                    nc.tensor.transpose(ptr[:], ocs[:], sb['eye'][0:96, 0:96])
                    nc.vector.tensor_copy(ot_all[:, sl, :], ptr[:])
                nc.gpsimd.dma_start(
                    yout[k01 * L:(k01 + 1) * L].rearrange('(a p) c -> p a c', p=128),
                    ot_all[:])


_CACHED = {}


def kernel(**inputs) -> np.ndarray:
    if 'nc' not in _CACHED:
        _CACHED['nc'] = build_program()
    nc = _CACHED['nc']
    w = _prep_weights(inputs)
    x = np.ascontiguousarray(np.asarray(inputs['x'], np.float32))
    in_maps = []
    for b in range(B):
        m = {'xin': np.ascontiguousarray(x[b, :2].reshape(NT, DM))}
        for name, _ in WSPECS:
            import ml_dtypes
            dt = ml_dtypes.bfloat16 if name in BF16_W else np.float32
            m[name] = np.ascontiguousarray(w[name].astype(dt))
        in_maps.append(m)
    res = run_bass_kernel_spmd(nc, in_maps, list(range(B)))
    out = np.stack([res.results[b]['yout'].reshape(2, H, W, DM) for b in range(B)])
    return out.astype(np.float32)

